# revision 46
# baseline (speedup 1.0000x reference)
"""AFNO2D layer on 8 TRN2 NeuronCores.

Sharding: channel-block parallel. Core i owns channels [96*i, 96*(i+1)) —
exactly block i of the block-diagonal MLP. No collectives.

v3: Hermitian forward DFT, contiguous-stationary S2 with paired single
drains, IH with contiguous moving operand, and cross-sample overlap
(sample b+1's S1/S2 fills the PE idle while sample b runs P1/IH/P2/IW).

Per core, per batch sample (tokens t = hk*65 + wc, NT = 8320):
  S1  H-DFT, kh=0..64 only (real input => Hermitian in kh).
      lhsT=x_c [h,w], rhs=fh2=[Ch|Sh] [128,130] -> psum [w, 130] per ch.
      Drain transposes into ZtT [w, khri(130), c] (strided DVE/ACT write)
      so S2's stationary loads are contiguous. Own PSUM pool (psC) so it
      can run while the previous sample's inverse phase occupies psA.
  S2  W-rDFT per kh-pair (k, 128-k): rows share the products Ztr@{Cw,Sw},
      Zti@{Sw,Cw}: 2 contiguous LDW + 2 MM N=260 per pair. One drain per
      pair via a step-sliced Xri view covering rows k and 128-k.
  L1  MLP layer 1 (bias via ones-row), relu drain; chunks of 7 hk,
      emitted as soon as their S2 pairs are done.
  L2  MLP layer 2, softshrink drain -> O2 [c, ri, wc, hk].
  P1  DMA transpose -> Y2 [hk, ri, wc, c]. Y2 lives inside the Xri
      buffer (Xri is fully consumed before P1 writes); the ones-row
      (partition 96) is re-DMAed after IH reads.
  IH  H-iDFT F-stationary, moving = wc-chunks of Y2 (contiguous),
      strided drain -> Z [h, ri, c, wcpad(128)].
  P2  DMA transpose c-eighths -> Zp [wcpad, ri, c12, h] (double-buffered)
  IW  W-irDFT F-stationary: lhsT=fwi2=[Cwi|-Swi] -> [w, 512]-chunks -> HBM
Residual add + final transpose run on the host in fp32.
"""
import sys
import types
import numpy as np
import ml_dtypes

# run_bass_kernel_spmd(trace=True) needs this hook module; missing in image.
if "antenv.axon_hooks" not in sys.modules:
    _hooks_mod = types.ModuleType("antenv.axon_hooks")
    _hooks_mod._hook = None
    _hooks_mod.set_axon_ntff_profile_hook = lambda h: setattr(_hooks_mod, "_hook", h)
    _hooks_mod.get_axon_ntff_profile_hook = lambda: _hooks_mod._hook
    sys.modules["antenv.axon_hooks"] = _hooks_mod
    try:
        sys.path.insert(0, "/root/.axon_site")
        from trn_agent_boot.trn_boot import _ntff_profile_via_ctypes
        _hooks_mod._hook = _ntff_profile_via_ctypes("/opt/axon/libaxon_pjrt.so")
    except Exception:
        pass

import concourse.bacc as bacc
import concourse.tile as tile
from concourse import mybir
from concourse.bass_utils import run_bass_kernel_spmd

F32 = mybir.dt.float32
BF16 = mybir.dt.bfloat16

B, H, W, C = 4, 128, 128, 768
Wc = W // 2 + 1            # 65
NCORES, BLK = 8, 96        # channels per core
NT = H * Wc                # 8320 tokens per sample
LAM = 0.01
CHK = 455                  # 7 hk per MLP chunk

_cache = {}


def _build_consts():
    bf = ml_dtypes.bfloat16
    h = np.arange(H)
    k65 = np.arange(Wc)
    wc = np.arange(Wc)
    w = np.arange(W)
    hk = np.arange(H)
    ang_h = 2 * np.pi * np.outer(h, k65) / H
    Ch, Sh = np.cos(ang_h) / np.sqrt(H), -np.sin(ang_h) / np.sqrt(H)
    ang_w = 2 * np.pi * np.outer(w, wc) / W
    Cw, Sw = np.cos(ang_w) / np.sqrt(W), -np.sin(ang_w) / np.sqrt(W)
    alpha = np.ones(Wc)
    alpha[1:64] = 2.0
    ang_wi = 2 * np.pi * np.outer(wc, w) / W
    Cwi = alpha[:, None] * np.cos(ang_wi) / np.sqrt(W)
    Swi = alpha[:, None] * np.sin(ang_wi) / np.sqrt(W)
    ang_hi = 2 * np.pi * np.outer(hk, h) / H
    Chi, Shi = np.cos(ang_hi) / np.sqrt(H), np.sin(ang_hi) / np.sqrt(H)

    fh2 = np.concatenate([Ch, Sh], axis=1).astype(bf)                  # [128,130]
    # S2 pair trick: psum = Ztr@fw4 + Zti@fw4i
    #   cols 0:130   -> row k      (r|i)
    #   cols 130:260 -> row 128-k  (r|i)
    fw4 = np.concatenate([Cw, Sw, Cw, Sw], axis=1).astype(bf)          # [128,260]
    fw4i = np.concatenate([-Sw, Cw, Sw, -Cw], axis=1).astype(bf)       # [128,260]
    # iW (final, real out): out = Cwi^T Zpr + (-Swi)^T Zpi
    fwi2 = np.concatenate([Cwi, -Swi], axis=1).astype(bf)              # [65,256]
    # iH (complex): Zr = Chi^T Yr - Shi^T Yi ; Zi = Shi^T Yr + Chi^T Yi
    fhi3 = np.concatenate([Chi, -Shi, Shi], axis=1).astype(bf)         # [128,384]
    ones = np.ones((1, 2 * NT), dtype=np.float32).astype(bf)           # [1,16640]
    return fh2, fw4, fw4i, fwi2, fhi3, ones


def _pack_mlp(w1, b1, w2, b2, blk):
    """[97, 384] packs: cols [Wr;br | -Wi;0 | Wi;bi | Wr;0]."""
    def pack(wr, wi, br, bi):
        p = np.zeros((97, 384), dtype=np.float32)
        p[:96, 0:96] = wr
        p[96, 0:96] = br
        p[:96, 96:192] = -wi
        p[:96, 192:288] = wi
        p[96, 192:288] = bi
        p[:96, 288:384] = wr
        return p
    w1p = pack(w1[0, blk], w1[1, blk], b1[0, blk], b1[1, blk])
    w2p = pack(w2[0, blk], w2[1, blk], b2[0, blk], b2[1, blk])
    return w1p, w2p


def _build_graph():
    nc = bacc.Bacc("TRN2", target_bir_lowering=False, debug=False,
                   num_devices=NCORES)

    # x is pre-transposed to [B, H, C, W] and pre-cast to bf16 on the host
    # so S1's stationary loads are contiguous (fast weight load) and the
    # per-sample load is 3.1MB instead of 6.3MB on the SWDGE path.
    x_ext = nc.dram_tensor("x", [B, H, BLK, W], BF16, kind="ExternalInput").ap()
    fh_ext = nc.dram_tensor("fh2", [128, 130], BF16, kind="ExternalInput").ap()
    fw_ext = nc.dram_tensor("fw4", [128, 260], BF16, kind="ExternalInput").ap()
    fwi_ext2 = nc.dram_tensor("fw4i", [128, 260], BF16, kind="ExternalInput").ap()
    fwi_ext = nc.dram_tensor("fwi2", [65, 256], BF16, kind="ExternalInput").ap()
    fhi_ext = nc.dram_tensor("fhi3", [128, 384], BF16, kind="ExternalInput").ap()
    w1_ext = nc.dram_tensor("w1p", [97, 384], F32, kind="ExternalInput").ap()
    w2_ext = nc.dram_tensor("w2p", [97, 384], F32, kind="ExternalInput").ap()
    on_ext = nc.dram_tensor("ones", [1, 2 * NT], BF16, kind="ExternalInput").ap()
    ml_ext = nc.dram_tensor("mlam", [96, 1], F32, kind="ExternalInput").ap()
    # device out: 24 chunks of [w, 512] over (c h)-flat; host reassembles
    out_ext = nc.dram_tensor("out", [B, 24, W, 512], BF16,
                             kind="ExternalOutput").ap()

    SUB = mybir.AluOpType.subtract
    MIN = mybir.AluOpType.min
    MAX = mybir.AluOpType.max
    RELU = mybir.ActivationFunctionType.Relu

    # L1/L2 chunk j is ready after S2 pair max-unit mu(j)
    # L1/L2 chunks (h0, hn). Row 64 is isolated as a tiny final chunk so
    # the only MLP work gated on S2 pair 64 is 65 tokens: the L2 tail that
    # delays P1 at each sample boundary shrinks accordingly. Rows 63 and
    # 65..71 are then ready at pair 63.
    chunks = ([(7 * j, 7) for j in range(9)] + [(63, 1)] +
              [(65 + 7 * j, 7) for j in range(9)] + [(64, 1)])
    ready = {}
    for h0, hn in chunks:
        rows = range(h0, h0 + hn)
        mu = max(r if r <= 64 else H - r for r in rows)
        ready.setdefault(mu, []).append((h0, hn))

    with tile.TileContext(nc) as tc:
        with (
            tc.tile_pool(name="consts", bufs=1) as cpool,
            tc.tile_pool(name="stat", bufs=1) as spool,
            tc.tile_pool(name="stg", bufs=2) as stg,      # clip staging
            tc.tile_pool(name="stg2", bufs=3) as stg2,    # out staging
            tc.tile_pool(name="psA", bufs=2, space="PSUM") as psA,  # IH/IW
            tc.tile_pool(name="psB", bufs=3, space="PSUM") as psB,  # L1/L2
            tc.tile_pool(name="psC", bufs=3, space="PSUM") as psC,  # S1/S2
        ):
            # ---- constants / weights to SBUF (once) ----
            fh2 = cpool.tile([128, 130], BF16, tag="fh2")
            nc.sync.dma_start(out=fh2, in_=fh_ext)
            fw4 = cpool.tile([128, 260], BF16, tag="fw4")
            nc.sync.dma_start(out=fw4, in_=fw_ext)
            fw4i = cpool.tile([128, 260], BF16, tag="fw4i")
            nc.sync.dma_start(out=fw4i, in_=fwi_ext2)
            fwi2 = cpool.tile([65, 256], BF16, tag="fwi2")
            nc.sync.dma_start(out=fwi2, in_=fwi_ext)
            fhi3 = cpool.tile([128, 384], BF16, tag="fhi3")
            nc.sync.dma_start(out=fhi3, in_=fhi_ext)
            w1p = cpool.tile([97, 384], BF16, tag="w1p")
            nc.gpsimd.dma_start(out=w1p, in_=w1_ext)      # casting DMA
            w2p = cpool.tile([97, 384], BF16, tag="w2p")
            nc.gpsimd.dma_start(out=w2p, in_=w2_ext)
            mlam = cpool.tile([96, 1], F32, tag="mlam")
            nc.sync.dma_start(out=mlam, in_=ml_ext)

            # W slices: lhsT [K, 96]
            W1ra = w1p[:, 0:96]          # [97, 96] row96 = b1r
            W1mi = w1p[0:96, 96:192]     # -Wi
            W1ib = w1p[:, 192:288]       # Wi ; b1i
            W1rb = w1p[0:96, 288:384]    # Wr
            W2ra = w2p[:, 0:96]
            W2mi = w2p[0:96, 96:192]
            W2ib = w2p[:, 192:288]
            W2rb = w2p[0:96, 288:384]

            # ---- static tiles (time-shared across stages/samples) ----
            X32f = spool.tile([128, W * BLK], BF16, tag="x32")
            X32 = X32f.rearrange("p (c w) -> p c w", c=BLK)
            ZtTf = spool.tile([128, 130 * BLK], BF16, tag="ztt")
            ZtT = ZtTf.rearrange("p (k c) -> p k c", c=BLK)        # [128,130,96]
            # Xri [97(+ones row), hk, ri, wc] shares its buffer with Y2
            # (P1 out, [hk, ri, wc, c]): L1 fully consumes Xri before P1
            # writes; the ones row (partition 96) is re-DMAed after IH.
            SHR = spool.tile([128, H * 130], BF16, tag="shr")
            Xri = SHR.rearrange("p (h r a) -> p h r a", h=H, r=2)  # [128,128,2,65]
            Y2 = SHR[:, 0:2 * Wc * BLK].rearrange(
                "p (r a c) -> p r a c", r=2, a=Wc)                 # [128,2,65,96]
            nc.sync.dma_start(out=Xri[96:97, :, :, :],
                              in_=on_ext[:, 0:H * 130])
            O1 = spool.tile([97, 2, 3, CHK], BF16, tag="o1")      # 3-chunk ring
            nc.sync.dma_start(out=O1[96:97, :, :, :],
                              in_=on_ext[:, 0:2 * 3 * CHK])
            # wc-major so P1's DMA-transpose input [96, hk] is contiguous
            O2 = spool.tile([96, 2, Wc, H], BF16, tag="o2")
            # iH out [h, ri, c, wcpad]: wcpad=128 for P2 transpose; pad cols
            # 65:128 are never read downstream (IW reads Zp partitions 0:65).
            Z = spool.tile([128, 2 * BLK * 128], BF16, tag="z")
            Z3 = Z.rearrange("p (r c a) -> p r c a", r=2, c=BLK)   # [128,2,96,128]
            Zp = spool.tile([128, 3, 2, 12, 128], BF16, tag="zp")  # 3 c-eighths
            Zpf = Zp.rearrange("p s r a b -> p s r (a b)")

            # alternate PSUM-drain engines (Pool cannot read PSUM)
            rr = [0]

            def drain(out, in_):
                rr[0] += 1
                if rr[0] % 2:
                    nc.vector.tensor_copy(out, in_)
                else:
                    nc.scalar.copy(out, in_)

            Zf = Z3.rearrange("p r c a -> p r (c a)")
            O2f = O2.rearrange("p r a b -> p r (a b)")

            def emit_load(b):
                nc.gpsimd.dma_start(out=X32f, in_=x_ext[b])

            def emit_s1_group(c0):
                p1 = psC.tile([128, 390], F32, tag="psC")
                for j in range(3):
                    nc.tensor.matmul(p1[:, j * 130:(j + 1) * 130],
                                     X32[:, c0 + j, :], fh2[:],
                                     start=True, stop=True)
                drain(ZtT[:, :, c0:c0 + 3],
                      p1.rearrange("p (c k) -> p k c", c=3))

            def emit_p1(half):
                # in [96, (wc.128hk)] -> out [128hk, wc, 96c]; split by wc
                # ranges (block-diagonal per wc) so IH can start sooner: the
                # first piece is small (wc 0:15) to minimize the latency to
                # IH chunk 0, which gates the whole inverse chain.
                # All transposes stay on the sync queue: concurrent
                # transposes from two HWDGE queues can deadlock the xbar.
                lo, hi = ((0, 15), (15, 35), (35, Wc))[half]
                sl = slice(lo * H, hi * H)
                nc.sync.dma_start(out=Y2[:, 0, lo:hi, :], in_=O2f[:, 0, sl],
                                  transpose=True)
                nc.sync.dma_start(out=Y2[:, 1, lo:hi, :], in_=O2f[:, 1, sl],
                                  transpose=True)

            def emit_ih_chunk(w0):
                wn = 5
                n = wn * BLK
                yr = Y2[:, 0, w0:w0 + wn, :]
                yi = Y2[:, 1, w0:w0 + wn, :]
                pzr = psA.tile([128, 480], F32, tag="psA")
                pzi = psA.tile([128, 480], F32, tag="psA")
                # same stationary (Chi) back-to-back across the two psums
                nc.tensor.matmul(pzr[:, :n], fhi3[:, 0:128], yr,
                                 start=True, stop=False)
                nc.tensor.matmul(pzi[:, :n], fhi3[:, 0:128], yi,
                                 start=True, stop=False)
                nc.tensor.matmul(pzr[:, :n], fhi3[:, 128:256], yi,
                                 start=False, stop=True)
                nc.tensor.matmul(pzi[:, :n], fhi3[:, 256:384], yr,
                                 start=False, stop=True)
                drain(Z3[:, 0, :, w0:w0 + wn],
                      pzr[:, :n].rearrange("p (a b) -> p b a", a=wn))
                drain(Z3[:, 1, :, w0:w0 + wn],
                      pzi[:, :n].rearrange("p (a b) -> p b a", a=wn))

            def emit_ones():
                # restore the ones row (P1 overwrote partition 96 of SHR);
                # SWDGE queue so it does not block P2 in the sync FIFO
                nc.gpsimd.dma_start(out=Xri[96:97, :, :, :],
                                    in_=on_ext[:, 0:H * 130])

            def emit_p2(e):
                s = e % 3
                nc.sync.dma_start(out=Zp[:, s, 0],
                                  in_=Zf[:, 0, e * 1536:(e + 1) * 1536],
                                  transpose=True)
                nc.sync.dma_start(out=Zp[:, s, 1],
                                  in_=Zf[:, 1, e * 1536:(e + 1) * 1536],
                                  transpose=True)

            def emit_iw(b, e):
                s = e % 3
                # 3 chunks of 512; first two share stationary loads
                p5a = psA.tile([128, 512], F32, tag="psA")
                p5b = psA.tile([128, 512], F32, tag="psA")
                sla, slb = slice(0, 512), slice(512, 1024)
                nc.tensor.matmul(p5a[:], fwi2[:, 0:128],
                                 Zpf[0:65, s, 0, sla], start=True, stop=False)
                nc.tensor.matmul(p5b[:], fwi2[:, 0:128],
                                 Zpf[0:65, s, 0, slb], start=True, stop=False)
                nc.tensor.matmul(p5a[:], fwi2[:, 128:256],
                                 Zpf[0:65, s, 1, sla], start=False, stop=True)
                nc.tensor.matmul(p5b[:], fwi2[:, 128:256],
                                 Zpf[0:65, s, 1, slb], start=False, stop=True)
                for p5, kk in ((p5a, 0), (p5b, 1)):
                    ot = stg2.tile([128, 512], BF16, tag="ot")
                    drain(ot, p5[:])
                    nc.scalar.dma_start(out=out_ext[b, e * 3 + kk], in_=ot)
                p5c = psA.tile([128, 512], F32, tag="psA")
                slc = slice(1024, 1536)
                nc.tensor.matmul(p5c[:], fwi2[:, 0:128],
                                 Zpf[0:65, s, 0, slc], start=True, stop=False)
                nc.tensor.matmul(p5c[:], fwi2[:, 128:256],
                                 Zpf[0:65, s, 1, slc], start=False, stop=True)
                ot = stg2.tile([128, 512], BF16, tag="ot")
                drain(ot, p5c[:])
                nc.scalar.dma_start(out=out_ext[b, e * 3 + 2], in_=ot)

            kchunk = [0]

            def emit_l_chunk(h0, hn):
                n = hn * Wc
                xr = Xri[0:97, h0:h0 + hn, 0, :]
                xi = Xri[0:97, h0:h0 + hn, 1, :]
                pr = psB.tile([96, CHK], F32, tag="psB")
                pi = psB.tile([96, CHK], F32, tag="psB")
                nc.tensor.matmul(pr[:, :n], W1ra, xr, start=True, stop=False)
                nc.tensor.matmul(pr[:, :n], W1mi, xi[0:96],
                                 start=False, stop=True)
                nc.tensor.matmul(pi[:, :n], W1ib, xr, start=True, stop=False)
                nc.tensor.matmul(pi[:, :n], W1rb, xi[0:96],
                                 start=False, stop=True)
                kr = kchunk[0] % 3
                kchunk[0] += 1
                nc.vector.tensor_scalar(O1[0:96, 0, kr, :n], pr[:, :n],
                                        0.0, None, MAX)
                nc.scalar.activation(O1[0:96, 1, kr, :n], pi[:, :n], RELU)

                # L2 on the chunk just produced
                o1r = O1[:, 0, kr, :n]
                o1i = O1[:, 1, kr, :n]
                qr = psB.tile([96, CHK], F32, tag="psB")
                qi = psB.tile([96, CHK], F32, tag="psB")
                nc.tensor.matmul(qr[:, :n], W2ra, o1r, start=True, stop=False)
                nc.tensor.matmul(qr[:, :n], W2mi, o1i[0:96],
                                 start=False, stop=True)
                nc.tensor.matmul(qi[:, :n], W2ib, o1r, start=True, stop=False)
                nc.tensor.matmul(qi[:, :n], W2rb, o1i[0:96],
                                 start=False, stop=True)
                # O2 dst written contiguously (wc-major); psum operands
                # read with (wc, hk)-permuted APs instead
                o2r = O2[:, 0, :, h0:h0 + hn]
                o2i = O2[:, 1, :, h0:h0 + hn]
                qrv = qr[:, :n].rearrange("p (a b) -> p b a", b=Wc)
                t1 = stg.tile([96, CHK], F32, tag="clip")
                t1v = t1[:, :n].rearrange("p (a b) -> p b a", b=Wc)
                # real: softshrink = y - clip(y) on DVE
                nc.vector.tensor_scalar(t1[:, :n], qr[:, :n], LAM, -LAM,
                                        MIN, MAX)
                nc.vector.tensor_tensor(o2r, qrv, t1v, SUB)
                # imag: relu(y-lam) - relu(-y-lam) on Act, sub on Pool
                sa = stg.tile([96, CHK], BF16, tag="sa")
                sb = stg.tile([96, CHK], BF16, tag="sb")
                sav = sa[:, :n].rearrange("p (a b) -> p b a", b=Wc)
                sbv = sb[:, :n].rearrange("p (a b) -> p b a", b=Wc)
                nc.scalar.activation(sa[:, :n], qi[:, :n], RELU, bias=mlam)
                nc.scalar.activation(sb[:, :n], qi[:, :n], RELU, bias=mlam,
                                     scale=-1.0)
                nc.gpsimd.tensor_tensor(o2i, sav, sbv, SUB)

            def emit_s2_pair(k):
                ps = psC.tile([96, 260], F32, tag="psC")
                nc.tensor.matmul(ps[:], ZtT[:, k, :], fw4[:],
                                 start=True, stop=False)
                nc.tensor.matmul(ps[:], ZtT[:, 65 + k, :], fw4i[:],
                                 start=False, stop=True)
                if 1 <= k <= 63:
                    # one drain for rows k and 128-k via step-slice
                    dst = Xri[0:96, k:129 - k:128 - 2 * k, :, :]
                    drain(dst, ps.rearrange("p (g r a) -> p g r a", g=2, r=2))
                else:
                    drain(Xri[0:96, k, :, :], ps[:, 0:130])

            # ---- software-pipelined schedule: sample b's inverse phase is
            # ---- emitted interleaved into sample b+1's forward phase so the
            # ---- in-order PE stream has fill work during DMA transposes.
            emit_load(0)
            # P2(e) issue pair -> eighths; IW(e) issue pair -> eighth.
            # Every IW has >=8 pairs of lead behind its P2 transpose, and
            # P2(e) is emitted after IW(e-3) (same Zp slot, ring of 3).
            p2_at = {0: [0, 1, 2], 12: [3], 20: [4], 28: [5], 36: [6],
                     44: [7]}
            iw_at = {12: 0, 20: 1, 28: 2, 36: 3, 44: 4, 52: 5, 58: 6, 63: 7}
            for b in range(B):
                if b > 0:
                    emit_p1(0)  # P1(b-1); Y2 reuses Xri(b-1) buffer
                ih_sched = {19 + i: w0 for i, w0 in
                            enumerate(range(0, Wc, 5))}  # groups 19..31
                for g, c0 in enumerate(range(0, BLK, 3)):
                    emit_s1_group(c0)
                    if b > 0 and g == 3:
                        emit_p1(1)
                    if b > 0 and g == 6:
                        emit_p1(2)
                    if b > 0 and g in ih_sched:
                        emit_ih_chunk(ih_sched[g])
                if b > 0:
                    emit_ones()
                for k in range(65):
                    emit_s2_pair(k)
                    if k == 20 and b + 1 < B:
                        emit_load(b + 1)
                    if b > 0 and k in iw_at:
                        emit_iw(b - 1, iw_at[k])
                    if b > 0 and k in p2_at:
                        for e in p2_at[k]:
                            emit_p2(e)
                    for h0, hn in ready.get(k, []):
                        emit_l_chunk(h0, hn)

            # tail: inverse phase of the last sample
            emit_p1(0)
            emit_p1(1)
            emit_p1(2)
            for w0 in range(0, Wc, 5):
                emit_ih_chunk(w0)
            emit_p2(0)
            emit_p2(1)
            emit_p2(2)
            for e in range(8):
                emit_iw(B - 1, e)
                if e + 3 < 8:
                    emit_p2(e + 3)

    nc.compile()
    return nc


def kernel(x, w1, b1, w2, b2):
    x = np.ascontiguousarray(x, dtype=np.float32)
    key = "nc"
    if key not in _cache:
        _cache[key] = _build_graph()
    nc = _cache[key]

    in_maps = make_in_maps(x, w1, b1, w2, b2)
    res = run_bass_kernel_spmd(nc, in_maps, core_ids=list(range(NCORES)))
    # device layout [B, 24, w, 512] -> [B, w, c, h] -> [B, h, w, c]
    parts = []
    for i in range(NCORES):
        r = np.asarray(res.results[i]["out"], dtype=np.float32)
        r = r.reshape(B, 24, W, 4, H).transpose(0, 4, 2, 1, 3)
        parts.append(r.reshape(B, H, W, BLK))
    corr = np.concatenate(parts, axis=3)
    return (corr + x).astype(np.float32)


def make_in_maps(x, w1, b1, w2, b2):
    fh2, fw4, fw4i, fwi2, fhi3, ones = _build_consts()
    in_maps = []
    for i in range(NCORES):
        w1p, w2p = _pack_mlp(w1, b1, w2, b2, i)
        in_maps.append({
            "x": np.ascontiguousarray(
                x[:, :, :, i * BLK:(i + 1) * BLK].transpose(0, 1, 3, 2)
            ).astype(ml_dtypes.bfloat16),
            "fh2": fh2, "fw4": fw4, "fw4i": fw4i, "fwi2": fwi2, "fhi3": fhi3,
            "w1p": w1p, "w2p": w2p, "ones": ones,
            "mlam": np.full((96, 1), -LAM, dtype=np.float32),
        })
    return in_maps


# revision 47
# speedup vs baseline: 1.1004x; 1.1004x over previous
"""AFNO2D layer on 8 TRN2 NeuronCores.

Sharding: channel-block parallel. Core i owns channels [96*i, 96*(i+1)) —
exactly block i of the block-diagonal MLP. No collectives.

v3: Hermitian forward DFT, contiguous-stationary S2 with paired single
drains, IH with contiguous moving operand, and cross-sample overlap
(sample b+1's S1/S2 fills the PE idle while sample b runs P1/IH/P2/IW).

Per core, per batch sample (tokens t = hk*65 + wc, NT = 8320):
  S1  H-DFT, kh=0..64 only (real input => Hermitian in kh).
      lhsT=x_c [h,w], rhs=fh2=[Ch|Sh] [128,130] -> psum [w, 130] per ch.
      Drain transposes into ZtT [w, khri(130), c] (strided DVE/ACT write)
      so S2's stationary loads are contiguous. Own PSUM pool (psC) so it
      can run while the previous sample's inverse phase occupies psA.
  S2  W-rDFT per kh-pair (k, 128-k): rows share the products Ztr@{Cw,Sw},
      Zti@{Sw,Cw}: 2 contiguous LDW + 2 MM N=260 per pair. One drain per
      pair via a step-sliced Xri view covering rows k and 128-k.
  L1  MLP layer 1 (bias via ones-row), relu drain; chunks of 7 hk,
      emitted as soon as their S2 pairs are done.
  L2  MLP layer 2, softshrink drain -> O2 [c, ri, wc, hk].
  P1  DMA transpose -> Y2 [hk, ri, wc, c]. Y2 lives inside the Xri
      buffer (Xri is fully consumed before P1 writes); the ones-row
      (partition 96) is re-DMAed after IH reads.
  IH  H-iDFT F-stationary, moving = wc-chunks of Y2 (contiguous),
      strided drain -> Z [h, ri, c, wcpad(128)].
  P2  DMA transpose c-eighths -> Zp [wcpad, ri, c12, h] (double-buffered)
  IW  W-irDFT F-stationary: lhsT=fwi2=[Cwi|-Swi] -> [w, 512]-chunks -> HBM
Residual add + final transpose run on the host in fp32.
"""
import sys
import types
import numpy as np
import ml_dtypes

# run_bass_kernel_spmd(trace=True) needs this hook module; missing in image.
if "antenv.axon_hooks" not in sys.modules:
    _hooks_mod = types.ModuleType("antenv.axon_hooks")
    _hooks_mod._hook = None
    _hooks_mod.set_axon_ntff_profile_hook = lambda h: setattr(_hooks_mod, "_hook", h)
    _hooks_mod.get_axon_ntff_profile_hook = lambda: _hooks_mod._hook
    sys.modules["antenv.axon_hooks"] = _hooks_mod
    try:
        sys.path.insert(0, "/root/.axon_site")
        from trn_agent_boot.trn_boot import _ntff_profile_via_ctypes
        _hooks_mod._hook = _ntff_profile_via_ctypes("/opt/axon/libaxon_pjrt.so")
    except Exception:
        pass

import concourse.bacc as bacc
import concourse.tile as tile
from concourse import mybir
from concourse.bass_utils import run_bass_kernel_spmd

F32 = mybir.dt.float32
BF16 = mybir.dt.bfloat16

B, H, W, C = 4, 128, 128, 768
Wc = W // 2 + 1            # 65
NCORES, BLK = 8, 96        # channels per core
NT = H * Wc                # 8320 tokens per sample
LAM = 0.01
CHK = 455                  # 7 hk per MLP chunk

_cache = {}


def _build_consts():
    bf = ml_dtypes.bfloat16
    h = np.arange(H)
    k65 = np.arange(Wc)
    wc = np.arange(Wc)
    w = np.arange(W)
    hk = np.arange(H)
    ang_h = 2 * np.pi * np.outer(h, k65) / H
    Ch, Sh = np.cos(ang_h) / np.sqrt(H), -np.sin(ang_h) / np.sqrt(H)
    ang_w = 2 * np.pi * np.outer(w, wc) / W
    Cw, Sw = np.cos(ang_w) / np.sqrt(W), -np.sin(ang_w) / np.sqrt(W)
    alpha = np.ones(Wc)
    alpha[1:64] = 2.0
    ang_wi = 2 * np.pi * np.outer(wc, w) / W
    Cwi = alpha[:, None] * np.cos(ang_wi) / np.sqrt(W)
    Swi = alpha[:, None] * np.sin(ang_wi) / np.sqrt(W)
    ang_hi = 2 * np.pi * np.outer(hk, h) / H
    Chi, Shi = np.cos(ang_hi) / np.sqrt(H), np.sin(ang_hi) / np.sqrt(H)

    fh2 = np.concatenate([Ch, Sh], axis=1).astype(bf)                  # [128,130]
    # S2 pair trick: psum = Ztr@fw4 + Zti@fw4i
    #   cols 0:130   -> row k      (r|i)
    #   cols 130:260 -> row 128-k  (r|i)
    fw4 = np.concatenate([Cw, Sw, Cw, Sw], axis=1).astype(bf)          # [128,260]
    fw4i = np.concatenate([-Sw, Cw, Sw, -Cw], axis=1).astype(bf)       # [128,260]
    # iW (final, real out): out = Cwi^T Zpr + (-Swi)^T Zpi
    fwi2 = np.concatenate([Cwi, -Swi], axis=1).astype(bf)              # [65,256]
    # iH (complex): Zr = Chi^T Yr - Shi^T Yi ; Zi = Shi^T Yr + Chi^T Yi
    fhi3 = np.concatenate([Chi, -Shi, Shi], axis=1).astype(bf)         # [128,384]
    ones = np.ones((1, 2 * NT), dtype=np.float32).astype(bf)           # [1,16640]
    return fh2, fw4, fw4i, fwi2, fhi3, ones


def _pack_mlp(w1, b1, w2, b2, blk):
    """[97, 384] packs: cols [Wr;br | -Wi;0 | Wi;bi | Wr;0]."""
    def pack(wr, wi, br, bi):
        p = np.zeros((97, 384), dtype=np.float32)
        p[:96, 0:96] = wr
        p[96, 0:96] = br
        p[:96, 96:192] = -wi
        p[:96, 192:288] = wi
        p[96, 192:288] = bi
        p[:96, 288:384] = wr
        return p
    w1p = pack(w1[0, blk], w1[1, blk], b1[0, blk], b1[1, blk])
    w2p = pack(w2[0, blk], w2[1, blk], b2[0, blk], b2[1, blk])
    return w1p, w2p


def _build_graph():
    nc = bacc.Bacc("TRN2", target_bir_lowering=False, debug=False,
                   num_devices=NCORES)

    # x is pre-transposed to [B, H, C, W] and pre-cast to bf16 on the host
    # so S1's stationary loads are contiguous (fast weight load) and the
    # per-sample load is 3.1MB instead of 6.3MB on the SWDGE path.
    x_ext = nc.dram_tensor("x", [B, H, BLK, W], BF16, kind="ExternalInput").ap()
    fh_ext = nc.dram_tensor("fh2", [128, 130], BF16, kind="ExternalInput").ap()
    fw_ext = nc.dram_tensor("fw4", [128, 260], BF16, kind="ExternalInput").ap()
    fwi_ext2 = nc.dram_tensor("fw4i", [128, 260], BF16, kind="ExternalInput").ap()
    fwi_ext = nc.dram_tensor("fwi2", [65, 256], BF16, kind="ExternalInput").ap()
    fhi_ext = nc.dram_tensor("fhi3", [128, 384], BF16, kind="ExternalInput").ap()
    w1_ext = nc.dram_tensor("w1p", [97, 384], F32, kind="ExternalInput").ap()
    w2_ext = nc.dram_tensor("w2p", [97, 384], F32, kind="ExternalInput").ap()
    on_ext = nc.dram_tensor("ones", [1, 2 * NT], BF16, kind="ExternalInput").ap()
    ml_ext = nc.dram_tensor("mlam", [96, 1], F32, kind="ExternalInput").ap()
    # device out: 24 chunks of [w, 512] over (c h)-flat; host reassembles
    out_ext = nc.dram_tensor("out", [B, 24, W, 512], BF16,
                             kind="ExternalOutput").ap()

    SUB = mybir.AluOpType.subtract
    MIN = mybir.AluOpType.min
    MAX = mybir.AluOpType.max
    RELU = mybir.ActivationFunctionType.Relu

    # L1/L2 chunk j is ready after S2 pair max-unit mu(j)
    # L1/L2 chunks (h0, hn). Row 64 is isolated as a tiny final chunk so
    # the only MLP work gated on S2 pair 64 is 65 tokens: the L2 tail that
    # delays P1 at each sample boundary shrinks accordingly. Rows 63 and
    # 65..71 are then ready at pair 63.
    chunks = ([(7 * j, 7) for j in range(9)] + [(63, 1)] +
              [(65 + 7 * j, 7) for j in range(9)] + [(64, 1)])
    ready = {}
    for h0, hn in chunks:
        rows = range(h0, h0 + hn)
        mu = max(r if r <= 64 else H - r for r in rows)
        ready.setdefault(mu, []).append((h0, hn))

    with tile.TileContext(nc) as tc:
        with (
            tc.tile_pool(name="consts", bufs=1) as cpool,
            tc.tile_pool(name="stat", bufs=1) as spool,
            tc.tile_pool(name="stg", bufs=2) as stg,      # clip staging
            tc.tile_pool(name="stg2", bufs=3) as stg2,    # out staging
            tc.tile_pool(name="psA", bufs=2, space="PSUM") as psA,  # IH/IW
            tc.tile_pool(name="psB", bufs=3, space="PSUM") as psB,  # L1/L2
            tc.tile_pool(name="psC", bufs=3, space="PSUM") as psC,  # S1/S2
        ):
            # ---- constants / weights to SBUF (once) ----
            fh2 = cpool.tile([128, 130], BF16, tag="fh2")
            nc.sync.dma_start(out=fh2, in_=fh_ext)
            fw4 = cpool.tile([128, 260], BF16, tag="fw4")
            nc.sync.dma_start(out=fw4, in_=fw_ext)
            fw4i = cpool.tile([128, 260], BF16, tag="fw4i")
            nc.sync.dma_start(out=fw4i, in_=fwi_ext2)
            fwi2 = cpool.tile([65, 256], BF16, tag="fwi2")
            nc.sync.dma_start(out=fwi2, in_=fwi_ext)
            fhi3 = cpool.tile([128, 384], BF16, tag="fhi3")
            nc.sync.dma_start(out=fhi3, in_=fhi_ext)
            w1p = cpool.tile([97, 384], BF16, tag="w1p")
            nc.gpsimd.dma_start(out=w1p, in_=w1_ext)      # casting DMA
            w2p = cpool.tile([97, 384], BF16, tag="w2p")
            nc.gpsimd.dma_start(out=w2p, in_=w2_ext)
            mlam = cpool.tile([96, 1], F32, tag="mlam")
            nc.sync.dma_start(out=mlam, in_=ml_ext)

            # W slices: lhsT [K, 96]
            W1ra = w1p[:, 0:96]          # [97, 96] row96 = b1r
            W1mi = w1p[0:96, 96:192]     # -Wi
            W1ib = w1p[:, 192:288]       # Wi ; b1i
            W1rb = w1p[0:96, 288:384]    # Wr
            W2ra = w2p[:, 0:96]
            W2mi = w2p[0:96, 96:192]
            W2ib = w2p[:, 192:288]
            W2rb = w2p[0:96, 288:384]

            # ---- static tiles (time-shared across stages/samples) ----
            X32f = spool.tile([128, W * BLK], BF16, tag="x32")
            X32 = X32f.rearrange("p (c w) -> p c w", c=BLK)
            ZtTf = spool.tile([128, 130 * BLK], BF16, tag="ztt")
            ZtT = ZtTf.rearrange("p (k c) -> p k c", c=BLK)        # [128,130,96]
            # Xri [97(+ones row), hk, ri, wc] shares its buffer with Y2
            # (P1 out, [hk, ri, wc, c]): L1 fully consumes Xri before P1
            # writes; the ones row (partition 96) is re-DMAed after IH.
            SHR = spool.tile([128, H * 130], BF16, tag="shr")
            Xri = SHR.rearrange("p (h r a) -> p h r a", h=H, r=2)  # [128,128,2,65]
            Y2 = SHR[:, 0:2 * Wc * BLK].rearrange(
                "p (r a c) -> p r a c", r=2, a=Wc)                 # [128,2,65,96]
            nc.sync.dma_start(out=Xri[96:97, :, :, :],
                              in_=on_ext[:, 0:H * 130])
            O1 = spool.tile([97, 2, 3, CHK], BF16, tag="o1")      # 3-chunk ring
            nc.sync.dma_start(out=O1[96:97, :, :, :],
                              in_=on_ext[:, 0:2 * 3 * CHK])
            # wc-major so P1's DMA-transpose input [96, hk] is contiguous
            O2 = spool.tile([96, 2, Wc, H], BF16, tag="o2")
            # iH out [h, ri, c, wcpad]: wcpad=128 for P2 transpose; pad cols
            # 65:128 are never read downstream (IW reads Zp partitions 0:65).
            Z = spool.tile([128, 2 * BLK * 128], BF16, tag="z")
            Z3 = Z.rearrange("p (r c a) -> p r c a", r=2, c=BLK)   # [128,2,96,128]
            Zp = spool.tile([128, 3, 2, 12, 128], BF16, tag="zp")  # 3 c-eighths
            Zpf = Zp.rearrange("p s r a b -> p s r (a b)")

            # alternate PSUM-drain engines (Pool cannot read PSUM)
            rr = [0]

            def drain(out, in_):
                rr[0] += 1
                if rr[0] % 2:
                    nc.vector.tensor_copy(out, in_)
                else:
                    nc.scalar.copy(out, in_)

            Zf = Z3.rearrange("p r c a -> p r (c a)")
            O2f = O2.rearrange("p r a b -> p r (a b)")

            def emit_load(b):
                nc.gpsimd.dma_start(out=X32f, in_=x_ext[b])

            def emit_s1_group(c0):
                p1 = psC.tile([128, 390], F32, tag="psC")
                for j in range(3):
                    nc.tensor.matmul(p1[:, j * 130:(j + 1) * 130],
                                     X32[:, c0 + j, :], fh2[:],
                                     start=True, stop=True)
                drain(ZtT[:, :, c0:c0 + 3],
                      p1.rearrange("p (c k) -> p k c", c=3))

            def emit_p1(half):
                # in [96, (wc.128hk)] -> out [128hk, wc, 96c]; split by wc
                # ranges (block-diagonal per wc) so IH can start sooner: the
                # first piece is small (wc 0:15) to minimize the latency to
                # IH chunk 0, which gates the whole inverse chain.
                # All transposes stay on the sync queue: concurrent
                # transposes from two HWDGE queues can deadlock the xbar.
                lo, hi = ((0, 15), (15, 35), (35, Wc))[half]
                sl = slice(lo * H, hi * H)
                nc.sync.dma_start(out=Y2[:, 0, lo:hi, :], in_=O2f[:, 0, sl],
                                  transpose=True)
                nc.sync.dma_start(out=Y2[:, 1, lo:hi, :], in_=O2f[:, 1, sl],
                                  transpose=True)

            def emit_ih_chunk(w0):
                wn = 5
                n = wn * BLK
                yr = Y2[:, 0, w0:w0 + wn, :]
                yi = Y2[:, 1, w0:w0 + wn, :]
                pzr = psA.tile([128, 480], F32, tag="psA")
                pzi = psA.tile([128, 480], F32, tag="psA")
                # same stationary (Chi) back-to-back across the two psums
                nc.tensor.matmul(pzr[:, :n], fhi3[:, 0:128], yr,
                                 start=True, stop=False)
                nc.tensor.matmul(pzi[:, :n], fhi3[:, 0:128], yi,
                                 start=True, stop=False)
                nc.tensor.matmul(pzr[:, :n], fhi3[:, 128:256], yi,
                                 start=False, stop=True)
                nc.tensor.matmul(pzi[:, :n], fhi3[:, 256:384], yr,
                                 start=False, stop=True)
                drain(Z3[:, 0, :, w0:w0 + wn],
                      pzr[:, :n].rearrange("p (a b) -> p b a", a=wn))
                drain(Z3[:, 1, :, w0:w0 + wn],
                      pzi[:, :n].rearrange("p (a b) -> p b a", a=wn))

            def emit_ones():
                # restore the ones row (P1 overwrote partition 96 of SHR);
                # SWDGE queue so it does not block P2 in the sync FIFO
                nc.gpsimd.dma_start(out=Xri[96:97, :, :, :],
                                    in_=on_ext[:, 0:H * 130])

            def emit_p2(e):
                s = e % 3
                nc.sync.dma_start(out=Zp[:, s, 0],
                                  in_=Zf[:, 0, e * 1536:(e + 1) * 1536],
                                  transpose=True)
                nc.sync.dma_start(out=Zp[:, s, 1],
                                  in_=Zf[:, 1, e * 1536:(e + 1) * 1536],
                                  transpose=True)

            def emit_iw(b, e):
                s = e % 3
                # 3 chunks of 512; first two share stationary loads
                p5a = psA.tile([128, 512], F32, tag="psA")
                p5b = psA.tile([128, 512], F32, tag="psA")
                sla, slb = slice(0, 512), slice(512, 1024)
                nc.tensor.matmul(p5a[:], fwi2[:, 0:128],
                                 Zpf[0:65, s, 0, sla], start=True, stop=False)
                nc.tensor.matmul(p5b[:], fwi2[:, 0:128],
                                 Zpf[0:65, s, 0, slb], start=True, stop=False)
                nc.tensor.matmul(p5a[:], fwi2[:, 128:256],
                                 Zpf[0:65, s, 1, sla], start=False, stop=True)
                nc.tensor.matmul(p5b[:], fwi2[:, 128:256],
                                 Zpf[0:65, s, 1, slb], start=False, stop=True)
                for p5, kk in ((p5a, 0), (p5b, 1)):
                    ot = stg2.tile([128, 512], BF16, tag="ot")
                    drain(ot, p5[:])
                    nc.sync.dma_start(out=out_ext[b, e * 3 + kk], in_=ot)
                p5c = psA.tile([128, 512], F32, tag="psA")
                slc = slice(1024, 1536)
                nc.tensor.matmul(p5c[:], fwi2[:, 0:128],
                                 Zpf[0:65, s, 0, slc], start=True, stop=False)
                nc.tensor.matmul(p5c[:], fwi2[:, 128:256],
                                 Zpf[0:65, s, 1, slc], start=False, stop=True)
                ot = stg2.tile([128, 512], BF16, tag="ot")
                drain(ot, p5c[:])
                nc.sync.dma_start(out=out_ext[b, e * 3 + 2], in_=ot)

            kchunk = [0]

            def emit_l_chunk(h0, hn):
                n = hn * Wc
                xr = Xri[0:97, h0:h0 + hn, 0, :]
                xi = Xri[0:97, h0:h0 + hn, 1, :]
                pr = psB.tile([96, CHK], F32, tag="psB")
                pi = psB.tile([96, CHK], F32, tag="psB")
                nc.tensor.matmul(pr[:, :n], W1ra, xr, start=True, stop=False)
                nc.tensor.matmul(pr[:, :n], W1mi, xi[0:96],
                                 start=False, stop=True)
                nc.tensor.matmul(pi[:, :n], W1ib, xr, start=True, stop=False)
                nc.tensor.matmul(pi[:, :n], W1rb, xi[0:96],
                                 start=False, stop=True)
                kr = kchunk[0] % 3
                kchunk[0] += 1
                nc.vector.tensor_scalar(O1[0:96, 0, kr, :n], pr[:, :n],
                                        0.0, None, MAX)
                nc.scalar.activation(O1[0:96, 1, kr, :n], pi[:, :n], RELU)

                # L2 on the chunk just produced
                o1r = O1[:, 0, kr, :n]
                o1i = O1[:, 1, kr, :n]
                qr = psB.tile([96, CHK], F32, tag="psB")
                qi = psB.tile([96, CHK], F32, tag="psB")
                nc.tensor.matmul(qr[:, :n], W2ra, o1r, start=True, stop=False)
                nc.tensor.matmul(qr[:, :n], W2mi, o1i[0:96],
                                 start=False, stop=True)
                nc.tensor.matmul(qi[:, :n], W2ib, o1r, start=True, stop=False)
                nc.tensor.matmul(qi[:, :n], W2rb, o1i[0:96],
                                 start=False, stop=True)
                # O2 dst written contiguously (wc-major); psum operands
                # read with (wc, hk)-permuted APs instead
                o2r = O2[:, 0, :, h0:h0 + hn]
                o2i = O2[:, 1, :, h0:h0 + hn]
                qrv = qr[:, :n].rearrange("p (a b) -> p b a", b=Wc)
                t1 = stg.tile([96, CHK], F32, tag="clip")
                t1v = t1[:, :n].rearrange("p (a b) -> p b a", b=Wc)
                # real: softshrink = y - clip(y) on DVE
                nc.vector.tensor_scalar(t1[:, :n], qr[:, :n], LAM, -LAM,
                                        MIN, MAX)
                nc.vector.tensor_tensor(o2r, qrv, t1v, SUB)
                # imag: relu(y-lam) - relu(-y-lam) on Act, sub on Pool
                sa = stg.tile([96, CHK], BF16, tag="sa")
                sb = stg.tile([96, CHK], BF16, tag="sb")
                sav = sa[:, :n].rearrange("p (a b) -> p b a", b=Wc)
                sbv = sb[:, :n].rearrange("p (a b) -> p b a", b=Wc)
                nc.scalar.activation(sa[:, :n], qi[:, :n], RELU, bias=mlam)
                nc.scalar.activation(sb[:, :n], qi[:, :n], RELU, bias=mlam,
                                     scale=-1.0)
                nc.gpsimd.tensor_tensor(o2i, sav, sbv, SUB)

            def emit_s2_pair(k):
                ps = psC.tile([96, 260], F32, tag="psC")
                nc.tensor.matmul(ps[:], ZtT[:, k, :], fw4[:],
                                 start=True, stop=False)
                nc.tensor.matmul(ps[:], ZtT[:, 65 + k, :], fw4i[:],
                                 start=False, stop=True)
                if 1 <= k <= 63:
                    # one drain for rows k and 128-k via step-slice
                    dst = Xri[0:96, k:129 - k:128 - 2 * k, :, :]
                    drain(dst, ps.rearrange("p (g r a) -> p g r a", g=2, r=2))
                else:
                    drain(Xri[0:96, k, :, :], ps[:, 0:130])

            # ---- software-pipelined schedule: sample b's inverse phase is
            # ---- emitted interleaved into sample b+1's forward phase so the
            # ---- in-order PE stream has fill work during DMA transposes.
            emit_load(0)
            # P2(e) issue pair -> eighths; IW(e) issue pair -> eighth.
            # Every IW has >=8 pairs of lead behind its P2 transpose, and
            # P2(e) is emitted after IW(e-3) (same Zp slot, ring of 3).
            p2_at = {0: [0, 1, 2], 12: [3], 20: [4], 28: [5], 36: [6],
                     44: [7]}
            iw_at = {12: 0, 20: 1, 28: 2, 36: 3, 44: 4, 52: 5, 58: 6, 63: 7}
            for b in range(B):
                if b > 0:
                    emit_p1(0)  # P1(b-1); Y2 reuses Xri(b-1) buffer
                ih_sched = {19 + i: w0 for i, w0 in
                            enumerate(range(0, Wc, 5))}  # groups 19..31
                for g, c0 in enumerate(range(0, BLK, 3)):
                    emit_s1_group(c0)
                    if b > 0 and g == 3:
                        emit_p1(1)
                    if b > 0 and g == 6:
                        emit_p1(2)
                    if b > 0 and g in ih_sched:
                        emit_ih_chunk(ih_sched[g])
                if b > 0:
                    emit_ones()
                for k in range(65):
                    emit_s2_pair(k)
                    if k == 20 and b + 1 < B:
                        emit_load(b + 1)
                    if b > 0 and k in iw_at:
                        emit_iw(b - 1, iw_at[k])
                    if b > 0 and k in p2_at:
                        for e in p2_at[k]:
                            emit_p2(e)
                    for h0, hn in ready.get(k, []):
                        emit_l_chunk(h0, hn)

            # tail: inverse phase of the last sample
            emit_p1(0)
            emit_p1(1)
            emit_p1(2)
            for w0 in range(0, Wc, 5):
                emit_ih_chunk(w0)
            emit_p2(0)
            emit_p2(1)
            emit_p2(2)
            for e in range(8):
                emit_iw(B - 1, e)
                if e + 3 < 8:
                    emit_p2(e + 3)

    nc.compile()
    return nc


def kernel(x, w1, b1, w2, b2):
    x = np.ascontiguousarray(x, dtype=np.float32)
    key = "nc"
    if key not in _cache:
        _cache[key] = _build_graph()
    nc = _cache[key]

    in_maps = make_in_maps(x, w1, b1, w2, b2)
    res = run_bass_kernel_spmd(nc, in_maps, core_ids=list(range(NCORES)))
    # device layout [B, 24, w, 512] -> [B, w, c, h] -> [B, h, w, c]
    parts = []
    for i in range(NCORES):
        r = np.asarray(res.results[i]["out"], dtype=np.float32)
        r = r.reshape(B, 24, W, 4, H).transpose(0, 4, 2, 1, 3)
        parts.append(r.reshape(B, H, W, BLK))
    corr = np.concatenate(parts, axis=3)
    return (corr + x).astype(np.float32)


def make_in_maps(x, w1, b1, w2, b2):
    fh2, fw4, fw4i, fwi2, fhi3, ones = _build_consts()
    in_maps = []
    for i in range(NCORES):
        w1p, w2p = _pack_mlp(w1, b1, w2, b2, i)
        in_maps.append({
            "x": np.ascontiguousarray(
                x[:, :, :, i * BLK:(i + 1) * BLK].transpose(0, 1, 3, 2)
            ).astype(ml_dtypes.bfloat16),
            "fh2": fh2, "fw4": fw4, "fw4i": fw4i, "fwi2": fwi2, "fhi3": fhi3,
            "w1p": w1p, "w2p": w2p, "ones": ones,
            "mlam": np.full((96, 1), -LAM, dtype=np.float32),
        })
    return in_maps


# revision 48
# speedup vs baseline: 1.1139x; 1.0123x over previous
"""AFNO2D layer on 8 TRN2 NeuronCores.

Sharding: channel-block parallel. Core i owns channels [96*i, 96*(i+1)) —
exactly block i of the block-diagonal MLP. No collectives.

v3: Hermitian forward DFT, contiguous-stationary S2 with paired single
drains, IH with contiguous moving operand, and cross-sample overlap
(sample b+1's S1/S2 fills the PE idle while sample b runs P1/IH/P2/IW).

Per core, per batch sample (tokens t = hk*65 + wc, NT = 8320):
  S1  H-DFT, kh=0..64 only (real input => Hermitian in kh).
      lhsT=x_c [h,w], rhs=fh2=[Ch|Sh] [128,130] -> psum [w, 130] per ch.
      Drain transposes into ZtT [w, khri(130), c] (strided DVE/ACT write)
      so S2's stationary loads are contiguous. Own PSUM pool (psC) so it
      can run while the previous sample's inverse phase occupies psA.
  S2  W-rDFT per kh-pair (k, 128-k): rows share the products Ztr@{Cw,Sw},
      Zti@{Sw,Cw}: 2 contiguous LDW + 2 MM N=260 per pair. One drain per
      pair via a step-sliced Xri view covering rows k and 128-k.
  L1  MLP layer 1 (bias via ones-row), relu drain; chunks of 7 hk,
      emitted as soon as their S2 pairs are done.
  L2  MLP layer 2, softshrink drain -> O2 [c, ri, wc, hk].
  P1  DMA transpose -> Y2 [hk, ri, wc, c]. Y2 lives inside the Xri
      buffer (Xri is fully consumed before P1 writes); the ones-row
      (partition 96) is re-DMAed after IH reads.
  IH  H-iDFT F-stationary, moving = wc-chunks of Y2 (contiguous),
      strided drain -> Z [h, ri, c, wcpad(128)].
  P2  DMA transpose c-eighths -> Zp [wcpad, ri, c12, h] (double-buffered)
  IW  W-irDFT F-stationary: lhsT=fwi2=[Cwi|-Swi] -> [w, 512]-chunks -> HBM
Residual add + final transpose run on the host in fp32.
"""
import sys
import types
import numpy as np
import ml_dtypes

# run_bass_kernel_spmd(trace=True) needs this hook module; missing in image.
if "antenv.axon_hooks" not in sys.modules:
    _hooks_mod = types.ModuleType("antenv.axon_hooks")
    _hooks_mod._hook = None
    _hooks_mod.set_axon_ntff_profile_hook = lambda h: setattr(_hooks_mod, "_hook", h)
    _hooks_mod.get_axon_ntff_profile_hook = lambda: _hooks_mod._hook
    sys.modules["antenv.axon_hooks"] = _hooks_mod
    try:
        sys.path.insert(0, "/root/.axon_site")
        from trn_agent_boot.trn_boot import _ntff_profile_via_ctypes
        _hooks_mod._hook = _ntff_profile_via_ctypes("/opt/axon/libaxon_pjrt.so")
    except Exception:
        pass

import concourse.bacc as bacc
import concourse.tile as tile
from concourse import mybir
from concourse.bass_utils import run_bass_kernel_spmd

F32 = mybir.dt.float32
BF16 = mybir.dt.bfloat16

B, H, W, C = 4, 128, 128, 768
Wc = W // 2 + 1            # 65
NCORES, BLK = 8, 96        # channels per core
NT = H * Wc                # 8320 tokens per sample
LAM = 0.01
CHK = 455                  # 7 hk per MLP chunk

_cache = {}


def _build_consts():
    bf = ml_dtypes.bfloat16
    h = np.arange(H)
    k65 = np.arange(Wc)
    wc = np.arange(Wc)
    w = np.arange(W)
    hk = np.arange(H)
    ang_h = 2 * np.pi * np.outer(h, k65) / H
    Ch, Sh = np.cos(ang_h) / np.sqrt(H), -np.sin(ang_h) / np.sqrt(H)
    ang_w = 2 * np.pi * np.outer(w, wc) / W
    Cw, Sw = np.cos(ang_w) / np.sqrt(W), -np.sin(ang_w) / np.sqrt(W)
    alpha = np.ones(Wc)
    alpha[1:64] = 2.0
    ang_wi = 2 * np.pi * np.outer(wc, w) / W
    Cwi = alpha[:, None] * np.cos(ang_wi) / np.sqrt(W)
    Swi = alpha[:, None] * np.sin(ang_wi) / np.sqrt(W)
    ang_hi = 2 * np.pi * np.outer(hk, h) / H
    Chi, Shi = np.cos(ang_hi) / np.sqrt(H), np.sin(ang_hi) / np.sqrt(H)

    fh2 = np.concatenate([Ch, Sh], axis=1).astype(bf)                  # [128,130]
    # S2 pair trick: psum = Ztr@fw4 + Zti@fw4i
    #   cols 0:130   -> row k      (r|i)
    #   cols 130:260 -> row 128-k  (r|i)
    fw4 = np.concatenate([Cw, Sw, Cw, Sw], axis=1).astype(bf)          # [128,260]
    fw4i = np.concatenate([-Sw, Cw, Sw, -Cw], axis=1).astype(bf)       # [128,260]
    # iW (final, real out): out = Cwi^T Zpr + (-Swi)^T Zpi
    fwi2 = np.concatenate([Cwi, -Swi], axis=1).astype(bf)              # [65,256]
    # iH (complex): Zr = Chi^T Yr - Shi^T Yi ; Zi = Shi^T Yr + Chi^T Yi
    fhi3 = np.concatenate([Chi, -Shi, Shi], axis=1).astype(bf)         # [128,384]
    ones = np.ones((1, 2 * NT), dtype=np.float32).astype(bf)           # [1,16640]
    return fh2, fw4, fw4i, fwi2, fhi3, ones


def _pack_mlp(w1, b1, w2, b2, blk):
    """[97, 384] packs: cols [Wr;br | -Wi;0 | Wi;bi | Wr;0]."""
    def pack(wr, wi, br, bi):
        p = np.zeros((97, 384), dtype=np.float32)
        p[:96, 0:96] = wr
        p[96, 0:96] = br
        p[:96, 96:192] = -wi
        p[:96, 192:288] = wi
        p[96, 192:288] = bi
        p[:96, 288:384] = wr
        return p
    w1p = pack(w1[0, blk], w1[1, blk], b1[0, blk], b1[1, blk])
    w2p = pack(w2[0, blk], w2[1, blk], b2[0, blk], b2[1, blk])
    return w1p, w2p


def _build_graph():
    nc = bacc.Bacc("TRN2", target_bir_lowering=False, debug=False,
                   num_devices=NCORES)

    # x is pre-transposed to [B, H, C, W] and pre-cast to bf16 on the host
    # so S1's stationary loads are contiguous (fast weight load) and the
    # per-sample load is 3.1MB instead of 6.3MB on the SWDGE path.
    x_ext = nc.dram_tensor("x", [B, H, BLK, W], BF16, kind="ExternalInput").ap()
    fh_ext = nc.dram_tensor("fh2", [128, 130], BF16, kind="ExternalInput").ap()
    fw_ext = nc.dram_tensor("fw4", [128, 260], BF16, kind="ExternalInput").ap()
    fwi_ext2 = nc.dram_tensor("fw4i", [128, 260], BF16, kind="ExternalInput").ap()
    fwi_ext = nc.dram_tensor("fwi2", [65, 256], BF16, kind="ExternalInput").ap()
    fhi_ext = nc.dram_tensor("fhi3", [128, 384], BF16, kind="ExternalInput").ap()
    w1_ext = nc.dram_tensor("w1p", [97, 384], F32, kind="ExternalInput").ap()
    w2_ext = nc.dram_tensor("w2p", [97, 384], F32, kind="ExternalInput").ap()
    on_ext = nc.dram_tensor("ones", [1, 2 * NT], BF16, kind="ExternalInput").ap()
    ml_ext = nc.dram_tensor("mlam", [96, 1], F32, kind="ExternalInput").ap()
    # device out: 24 chunks of [w, 512] over (c h)-flat; host reassembles
    out_ext = nc.dram_tensor("out", [B, 24, W, 512], BF16,
                             kind="ExternalOutput").ap()

    SUB = mybir.AluOpType.subtract
    MIN = mybir.AluOpType.min
    MAX = mybir.AluOpType.max
    RELU = mybir.ActivationFunctionType.Relu

    # L1/L2 chunk j is ready after S2 pair max-unit mu(j)
    # L1/L2 chunks (h0, hn). Row 64 is isolated as a tiny final chunk so
    # the only MLP work gated on S2 pair 64 is 65 tokens: the L2 tail that
    # delays P1 at each sample boundary shrinks accordingly. Rows 63 and
    # 65..71 are then ready at pair 63.
    chunks = ([(7 * j, 7) for j in range(9)] + [(63, 1)] +
              [(65 + 7 * j, 7) for j in range(9)] + [(64, 1)])
    ready = {}
    for h0, hn in chunks:
        rows = range(h0, h0 + hn)
        mu = max(r if r <= 64 else H - r for r in rows)
        ready.setdefault(mu, []).append((h0, hn))

    with tile.TileContext(nc) as tc:
        with (
            tc.tile_pool(name="consts", bufs=1) as cpool,
            tc.tile_pool(name="stat", bufs=1) as spool,
            tc.tile_pool(name="stg", bufs=2) as stg,      # clip staging
            tc.tile_pool(name="stg2", bufs=3) as stg2,    # out staging
            tc.tile_pool(name="psA", bufs=2, space="PSUM") as psA,  # IH/IW
            tc.tile_pool(name="psB", bufs=3, space="PSUM") as psB,  # L1/L2
            tc.tile_pool(name="psC", bufs=3, space="PSUM") as psC,  # S1/S2
        ):
            # ---- constants / weights to SBUF (once) ----
            fh2 = cpool.tile([128, 130], BF16, tag="fh2")
            nc.sync.dma_start(out=fh2, in_=fh_ext)
            fw4 = cpool.tile([128, 260], BF16, tag="fw4")
            nc.sync.dma_start(out=fw4, in_=fw_ext)
            fw4i = cpool.tile([128, 260], BF16, tag="fw4i")
            nc.sync.dma_start(out=fw4i, in_=fwi_ext2)
            fwi2 = cpool.tile([65, 256], BF16, tag="fwi2")
            nc.sync.dma_start(out=fwi2, in_=fwi_ext)
            fhi3 = cpool.tile([128, 384], BF16, tag="fhi3")
            nc.sync.dma_start(out=fhi3, in_=fhi_ext)
            w1p = cpool.tile([97, 384], BF16, tag="w1p")
            nc.gpsimd.dma_start(out=w1p, in_=w1_ext)      # casting DMA
            w2p = cpool.tile([97, 384], BF16, tag="w2p")
            nc.gpsimd.dma_start(out=w2p, in_=w2_ext)
            mlam = cpool.tile([96, 1], F32, tag="mlam")
            nc.sync.dma_start(out=mlam, in_=ml_ext)

            # W slices: lhsT [K, 96]
            W1ra = w1p[:, 0:96]          # [97, 96] row96 = b1r
            W1mi = w1p[0:96, 96:192]     # -Wi
            W1ib = w1p[:, 192:288]       # Wi ; b1i
            W1rb = w1p[0:96, 288:384]    # Wr
            W2ra = w2p[:, 0:96]
            W2mi = w2p[0:96, 96:192]
            W2ib = w2p[:, 192:288]
            W2rb = w2p[0:96, 288:384]

            # ---- static tiles (time-shared across stages/samples) ----
            X32f = spool.tile([128, W * BLK], BF16, tag="x32")
            X32 = X32f.rearrange("p (c w) -> p c w", c=BLK)
            ZtTf = spool.tile([128, 130 * BLK], BF16, tag="ztt")
            ZtT = ZtTf.rearrange("p (k c) -> p k c", c=BLK)        # [128,130,96]
            # Xri [97(+ones row), hk, ri, wc] shares its buffer with Y2
            # (P1 out, [hk, ri, wc, c]): L1 fully consumes Xri before P1
            # writes; the ones row (partition 96) is re-DMAed after IH.
            SHR = spool.tile([128, H * 130], BF16, tag="shr")
            Xri = SHR.rearrange("p (h r a) -> p h r a", h=H, r=2)  # [128,128,2,65]
            Y2 = SHR[:, 0:2 * Wc * BLK].rearrange(
                "p (r a c) -> p r a c", r=2, a=Wc)                 # [128,2,65,96]
            nc.sync.dma_start(out=Xri[96:97, :, :, :],
                              in_=on_ext[:, 0:H * 130])
            O1 = spool.tile([97, 2, 3, CHK], BF16, tag="o1")      # 3-chunk ring
            nc.sync.dma_start(out=O1[96:97, :, :, :],
                              in_=on_ext[:, 0:2 * 3 * CHK])
            # wc-major so P1's DMA-transpose input [96, hk] is contiguous
            O2 = spool.tile([96, 2, Wc, H], BF16, tag="o2")
            # iH out [h, ri, c, wcpad]: wcpad=128 for P2 transpose; pad cols
            # 65:128 are never read downstream (IW reads Zp partitions 0:65).
            Z = spool.tile([128, 2 * BLK * 128], BF16, tag="z")
            Z3 = Z.rearrange("p (r c a) -> p r c a", r=2, c=BLK)   # [128,2,96,128]
            Zp = spool.tile([128, 4, 2, 12, 128], BF16, tag="zp")  # 4 c-eighths
            Zpf = Zp.rearrange("p s r a b -> p s r (a b)")

            # alternate PSUM-drain engines (Pool cannot read PSUM)
            rr = [0]

            def drain(out, in_):
                rr[0] += 1
                if rr[0] % 2:
                    nc.vector.tensor_copy(out, in_)
                else:
                    nc.scalar.copy(out, in_)

            Zf = Z3.rearrange("p r c a -> p r (c a)")
            O2f = O2.rearrange("p r a b -> p r (a b)")

            def emit_load(b):
                nc.gpsimd.dma_start(out=X32f, in_=x_ext[b])

            def emit_s1_group(c0):
                p1 = psC.tile([128, 390], F32, tag="psC")
                for j in range(3):
                    nc.tensor.matmul(p1[:, j * 130:(j + 1) * 130],
                                     X32[:, c0 + j, :], fh2[:],
                                     start=True, stop=True)
                drain(ZtT[:, :, c0:c0 + 3],
                      p1.rearrange("p (c k) -> p k c", c=3))

            def emit_p1(half):
                # in [96, (wc.128hk)] -> out [128hk, wc, 96c]; split by wc
                # ranges (block-diagonal per wc) so IH can start sooner: the
                # first piece is small (wc 0:15) to minimize the latency to
                # IH chunk 0, which gates the whole inverse chain.
                # All transposes stay on the sync queue: concurrent
                # transposes from two HWDGE queues can deadlock the xbar.
                lo, hi = ((0, 15), (15, 35), (35, Wc))[half]
                sl = slice(lo * H, hi * H)
                nc.sync.dma_start(out=Y2[:, 0, lo:hi, :], in_=O2f[:, 0, sl],
                                  transpose=True)
                nc.sync.dma_start(out=Y2[:, 1, lo:hi, :], in_=O2f[:, 1, sl],
                                  transpose=True)

            def emit_ih_chunk(w0):
                wn = 5
                n = wn * BLK
                yr = Y2[:, 0, w0:w0 + wn, :]
                yi = Y2[:, 1, w0:w0 + wn, :]
                pzr = psA.tile([128, 480], F32, tag="psA")
                pzi = psA.tile([128, 480], F32, tag="psA")
                # same stationary (Chi) back-to-back across the two psums
                nc.tensor.matmul(pzr[:, :n], fhi3[:, 0:128], yr,
                                 start=True, stop=False)
                nc.tensor.matmul(pzi[:, :n], fhi3[:, 0:128], yi,
                                 start=True, stop=False)
                nc.tensor.matmul(pzr[:, :n], fhi3[:, 128:256], yi,
                                 start=False, stop=True)
                nc.tensor.matmul(pzi[:, :n], fhi3[:, 256:384], yr,
                                 start=False, stop=True)
                drain(Z3[:, 0, :, w0:w0 + wn],
                      pzr[:, :n].rearrange("p (a b) -> p b a", a=wn))
                drain(Z3[:, 1, :, w0:w0 + wn],
                      pzi[:, :n].rearrange("p (a b) -> p b a", a=wn))

            def emit_ones():
                # restore the ones row (P1 overwrote partition 96 of SHR);
                # SWDGE queue so it does not block P2 in the sync FIFO
                nc.gpsimd.dma_start(out=Xri[96:97, :, :, :],
                                    in_=on_ext[:, 0:H * 130])

            def emit_p2(e):
                s = e % 4
                nc.sync.dma_start(out=Zp[:, s, 0],
                                  in_=Zf[:, 0, e * 1536:(e + 1) * 1536],
                                  transpose=True)
                nc.sync.dma_start(out=Zp[:, s, 1],
                                  in_=Zf[:, 1, e * 1536:(e + 1) * 1536],
                                  transpose=True)

            def emit_iw(b, e):
                s = e % 4
                # 3 chunks of 512; first two share stationary loads
                p5a = psA.tile([128, 512], F32, tag="psA")
                p5b = psA.tile([128, 512], F32, tag="psA")
                sla, slb = slice(0, 512), slice(512, 1024)
                nc.tensor.matmul(p5a[:], fwi2[:, 0:128],
                                 Zpf[0:65, s, 0, sla], start=True, stop=False)
                nc.tensor.matmul(p5b[:], fwi2[:, 0:128],
                                 Zpf[0:65, s, 0, slb], start=True, stop=False)
                nc.tensor.matmul(p5a[:], fwi2[:, 128:256],
                                 Zpf[0:65, s, 1, sla], start=False, stop=True)
                nc.tensor.matmul(p5b[:], fwi2[:, 128:256],
                                 Zpf[0:65, s, 1, slb], start=False, stop=True)
                for p5, kk in ((p5a, 0), (p5b, 1)):
                    ot = stg2.tile([128, 512], BF16, tag="ot")
                    drain(ot, p5[:])
                    nc.sync.dma_start(out=out_ext[b, e * 3 + kk], in_=ot)
                p5c = psA.tile([128, 512], F32, tag="psA")
                slc = slice(1024, 1536)
                nc.tensor.matmul(p5c[:], fwi2[:, 0:128],
                                 Zpf[0:65, s, 0, slc], start=True, stop=False)
                nc.tensor.matmul(p5c[:], fwi2[:, 128:256],
                                 Zpf[0:65, s, 1, slc], start=False, stop=True)
                ot = stg2.tile([128, 512], BF16, tag="ot")
                drain(ot, p5c[:])
                nc.sync.dma_start(out=out_ext[b, e * 3 + 2], in_=ot)

            kchunk = [0]

            def emit_l_chunk(h0, hn):
                n = hn * Wc
                xr = Xri[0:97, h0:h0 + hn, 0, :]
                xi = Xri[0:97, h0:h0 + hn, 1, :]
                pr = psB.tile([96, CHK], F32, tag="psB")
                pi = psB.tile([96, CHK], F32, tag="psB")
                nc.tensor.matmul(pr[:, :n], W1ra, xr, start=True, stop=False)
                nc.tensor.matmul(pr[:, :n], W1mi, xi[0:96],
                                 start=False, stop=True)
                nc.tensor.matmul(pi[:, :n], W1ib, xr, start=True, stop=False)
                nc.tensor.matmul(pi[:, :n], W1rb, xi[0:96],
                                 start=False, stop=True)
                kr = kchunk[0] % 3
                kchunk[0] += 1
                nc.vector.tensor_scalar(O1[0:96, 0, kr, :n], pr[:, :n],
                                        0.0, None, MAX)
                nc.scalar.activation(O1[0:96, 1, kr, :n], pi[:, :n], RELU)

                # L2 on the chunk just produced
                o1r = O1[:, 0, kr, :n]
                o1i = O1[:, 1, kr, :n]
                qr = psB.tile([96, CHK], F32, tag="psB")
                qi = psB.tile([96, CHK], F32, tag="psB")
                nc.tensor.matmul(qr[:, :n], W2ra, o1r, start=True, stop=False)
                nc.tensor.matmul(qr[:, :n], W2mi, o1i[0:96],
                                 start=False, stop=True)
                nc.tensor.matmul(qi[:, :n], W2ib, o1r, start=True, stop=False)
                nc.tensor.matmul(qi[:, :n], W2rb, o1i[0:96],
                                 start=False, stop=True)
                # O2 dst written contiguously (wc-major); psum operands
                # read with (wc, hk)-permuted APs instead
                o2r = O2[:, 0, :, h0:h0 + hn]
                o2i = O2[:, 1, :, h0:h0 + hn]
                qrv = qr[:, :n].rearrange("p (a b) -> p b a", b=Wc)
                t1 = stg.tile([96, CHK], F32, tag="clip")
                t1v = t1[:, :n].rearrange("p (a b) -> p b a", b=Wc)
                # real: softshrink = y - clip(y) on DVE
                nc.vector.tensor_scalar(t1[:, :n], qr[:, :n], LAM, -LAM,
                                        MIN, MAX)
                nc.vector.tensor_tensor(o2r, qrv, t1v, SUB)
                # imag: relu(y-lam) - relu(-y-lam) on Act, sub on Pool
                sa = stg.tile([96, CHK], BF16, tag="sa")
                sb = stg.tile([96, CHK], BF16, tag="sb")
                sav = sa[:, :n].rearrange("p (a b) -> p b a", b=Wc)
                sbv = sb[:, :n].rearrange("p (a b) -> p b a", b=Wc)
                nc.scalar.activation(sa[:, :n], qi[:, :n], RELU, bias=mlam)
                nc.scalar.activation(sb[:, :n], qi[:, :n], RELU, bias=mlam,
                                     scale=-1.0)
                nc.gpsimd.tensor_tensor(o2i, sav, sbv, SUB)

            def emit_s2_pair(k):
                ps = psC.tile([96, 260], F32, tag="psC")
                nc.tensor.matmul(ps[:], ZtT[:, k, :], fw4[:],
                                 start=True, stop=False)
                nc.tensor.matmul(ps[:], ZtT[:, 65 + k, :], fw4i[:],
                                 start=False, stop=True)
                if 1 <= k <= 63:
                    # one drain for rows k and 128-k via step-slice
                    dst = Xri[0:96, k:129 - k:128 - 2 * k, :, :]
                    drain(dst, ps.rearrange("p (g r a) -> p g r a", g=2, r=2))
                else:
                    drain(Xri[0:96, k, :, :], ps[:, 0:130])

            # ---- software-pipelined schedule: sample b's inverse phase is
            # ---- emitted interleaved into sample b+1's forward phase so the
            # ---- in-order PE stream has fill work during DMA transposes.
            emit_load(0)
            # P2(e) issue pair -> eighths; IW(e) issue pair -> eighth.
            # Every IW has >=8 pairs of lead behind its P2 transpose, and
            # P2(e) is emitted after IW(e-3) (same Zp slot, ring of 3).
            p2_at = {0: [0, 1, 2, 3], 12: [4], 20: [5], 28: [6],
                     36: [7]}
            iw_at = {12: 0, 20: 1, 28: 2, 36: 3, 44: 4, 52: 5, 58: 6, 63: 7}
            for b in range(B):
                if b > 0:
                    emit_p1(0)  # P1(b-1); Y2 reuses Xri(b-1) buffer
                ih_sched = {19 + i: w0 for i, w0 in
                            enumerate(range(0, Wc, 5))}  # groups 19..31
                for g, c0 in enumerate(range(0, BLK, 3)):
                    emit_s1_group(c0)
                    if b > 0 and g == 3:
                        emit_p1(1)
                    if b > 0 and g == 6:
                        emit_p1(2)
                    if b > 0 and g in ih_sched:
                        emit_ih_chunk(ih_sched[g])
                if b > 0:
                    emit_ones()
                for k in range(65):
                    emit_s2_pair(k)
                    if k == 20 and b + 1 < B:
                        emit_load(b + 1)
                    if b > 0 and k in iw_at:
                        emit_iw(b - 1, iw_at[k])
                    if b > 0 and k in p2_at:
                        for e in p2_at[k]:
                            emit_p2(e)
                    for h0, hn in ready.get(k, []):
                        emit_l_chunk(h0, hn)

            # tail: inverse phase of the last sample
            emit_p1(0)
            emit_p1(1)
            emit_p1(2)
            for w0 in range(0, Wc, 5):
                emit_ih_chunk(w0)
            emit_p2(0)
            emit_p2(1)
            emit_p2(2)
            emit_p2(3)
            for e in range(8):
                emit_iw(B - 1, e)
                if e + 4 < 8:
                    emit_p2(e + 4)

    nc.compile()
    return nc


def kernel(x, w1, b1, w2, b2):
    x = np.ascontiguousarray(x, dtype=np.float32)
    key = "nc"
    if key not in _cache:
        _cache[key] = _build_graph()
    nc = _cache[key]

    in_maps = make_in_maps(x, w1, b1, w2, b2)
    res = run_bass_kernel_spmd(nc, in_maps, core_ids=list(range(NCORES)))
    # device layout [B, 24, w, 512] -> [B, w, c, h] -> [B, h, w, c]
    parts = []
    for i in range(NCORES):
        r = np.asarray(res.results[i]["out"], dtype=np.float32)
        r = r.reshape(B, 24, W, 4, H).transpose(0, 4, 2, 1, 3)
        parts.append(r.reshape(B, H, W, BLK))
    corr = np.concatenate(parts, axis=3)
    return (corr + x).astype(np.float32)


def make_in_maps(x, w1, b1, w2, b2):
    fh2, fw4, fw4i, fwi2, fhi3, ones = _build_consts()
    in_maps = []
    for i in range(NCORES):
        w1p, w2p = _pack_mlp(w1, b1, w2, b2, i)
        in_maps.append({
            "x": np.ascontiguousarray(
                x[:, :, :, i * BLK:(i + 1) * BLK].transpose(0, 1, 3, 2)
            ).astype(ml_dtypes.bfloat16),
            "fh2": fh2, "fw4": fw4, "fw4i": fw4i, "fwi2": fwi2, "fhi3": fhi3,
            "w1p": w1p, "w2p": w2p, "ones": ones,
            "mlam": np.full((96, 1), -LAM, dtype=np.float32),
        })
    return in_maps
